# revision 10
# baseline (speedup 1.0000x reference)
"""Multi-head dilated sliding-window attention (window=129, dil=1) on 8 TRN2 cores.

Sharding: sequence-parallel. Each core computes 256 query rows (N=2048 / 8),
with a 64-row K/V halo on each side (zero-padded at the sequence edges).
Weights are replicated (resident in SBUF, bf16).

Band-softmax identity (reference softmaxes the FULL row with zeros outside
the band):
    out_i = (sum_band (e^{s_ij} - 1) V_j + sum_all V_j) / (sum_band (e^{s_ij} - 1) + N)
with V_raw = x@Wv (bv folded into bo' = bo + bv@Wo on the host), bk applied
only to real (non-padding) K rows via an indicator-row matmul, and the global
row  [sum_all V_j | N]  precomputed on the host (biascat).

v2 structure:
  - x arrives pre-transposed from the host (xT layout) -> no device transposes
  - V projection et-outer: starts as soon as wv tile 0 lands
  - per-head-pair rounds: Q/K proj -> scores -> exp/-1/mask (one fused DVE op)
    -> PV of the previous round, normalized A written to Asc (bf16)
  - Asc -> AT via per-round SBUF->SBUF DMA transposes (no PE transposes)
  - output projection accumulates from AT at the end; out DMA'd as bf16
"""

import numpy as np
import ml_dtypes
from contextlib import ExitStack

import concourse.bass as bass
import concourse.tile as tile
from concourse import bacc, mybir
from concourse.bass_utils import run_bass_kernel_spmd

F32 = mybir.dt.float32
BF16 = mybir.dt.bfloat16
NPBF16 = ml_dtypes.bfloat16
N, E, H, D = 2048, 1024, 16, 64
R = N // 8          # 256 query rows per core
HALO = R + 128      # 384 K/V rows per core
NQB = R // 128      # query blocks per core


def build_graph():
    nc = bacc.Bacc("TRN2", target_bir_lowering=False, debug=False, num_devices=8)

    xt_d = nc.declare_dram_parameter("xhT", [E, HALO], BF16, isOutput=False)
    xvalid_d = nc.declare_dram_parameter("xvalid", [1, HALO], BF16, isOutput=False)
    # Wq/Wk arrive db-major from the host: [db, e_part, et*128+d] so round db
    # depends on a single 256KB tile instead of the whole matrix.
    wq_d = nc.declare_dram_parameter("Wq_db", [8, 128, H * D], BF16,
                                     isOutput=False)
    wk_d = nc.declare_dram_parameter("Wk_db", [8, 128, H * D], BF16,
                                     isOutput=False)
    wv_d = nc.declare_dram_parameter("Wv", [E, H * D], BF16, isOutput=False)
    wo_d = nc.declare_dram_parameter("Wo", [H * D, E], BF16, isOutput=False)
    bq_d = nc.declare_dram_parameter("bq_r", [128, 8], F32, isOutput=False)
    bk_d = nc.declare_dram_parameter("bk_row", [1, H * D], BF16, isOutput=False)
    bo_d = nc.declare_dram_parameter("bo_row", [1, E], BF16, isOutput=False)
    bc_d = nc.declare_dram_parameter("biascat_r", [1, H * (D + 1)], BF16,
                                     isOutput=False)
    m4_d = nc.declare_dram_parameter("mask4", [128, 512], BF16, isOutput=False)
    out_d = nc.declare_dram_parameter("out", [R, E], BF16, isOutput=True)

    with tile.TileContext(nc) as tc, ExitStack() as ctx:
        const = ctx.enter_context(tc.tile_pool(name="const", bufs=1))
        pers = ctx.enter_context(tc.tile_pool(name="pers", bufs=1))
        epool = ctx.enter_context(tc.tile_pool(name="epool", bufs=3))
        ppool = ctx.enter_context(tc.tile_pool(name="ppool", bufs=5))
        zpool = ctx.enter_context(tc.tile_pool(name="zpool", bufs=4))
        obpool = ctx.enter_context(tc.tile_pool(name="obpool", bufs=2))
        psum = ctx.enter_context(tc.tile_pool(name="psum", bufs=8, space="PSUM"))

        def ps(shape, dt=F32):
            return psum.tile(shape, dt, tag="ps", name="pst")

        # ---- PE warm-up emitted first: dummy matmuls keep the PE busy while
        # the first DMAs land, so HAM is at 8/8 when real work starts.
        wu = const.tile([128, 512], BF16, tag="wu")
        nc.vector.memset(wu[:], 0.0)
        wups = psum.tile([128, 512], F32, tag="ps", name="wups")
        for _ in range(14):
            nc.tensor.matmul(wups[:], wu[:, 0:128], wu[:], start=True, stop=True)

        # ---- loads across three independent DMA rings:
        #   sync   (HWDGE): xT, Wv, then Wq/Wk db-pairs 1..7  (critical path)
        #   gpsimd (SWDGE): db-pair 0, small consts, Wo       (early + late)
        #   scalar (HWDGE): per-round Asc->AT transposes + output stores
        xT = pers.tile([128, 8, HALO], BF16, tag="xT")       # [e_p, e_t, seq]
        for et in range(8):
            nc.sync.dma_start(xT[:, et, :], xt_d[et * 128:(et + 1) * 128, :])

        def wtile(nm):
            return const.tile([128, E], BF16, tag=nm, name="wt")

        wv_t = [wtile(f"wv{et}") for et in range(8)]
        wq_t = [wtile(f"wq{db}") for db in range(8)]   # [e_p, et, d] per db
        wk_t = [wtile(f"wk{db}") for db in range(8)]
        wo_t = [wtile(f"wo{et}") for et in range(8)]
        m4 = const.tile([128, 512], BF16, tag="m4")
        bq_sb = const.tile([128, 8], F32, tag="bq")
        bk_sb = const.tile([1, H * D], BF16, tag="bk")
        bo_sb = const.tile([1, E], BF16, tag="bo")
        bc_sb = const.tile([1, H, D + 1], BF16, tag="bc")
        valid_sb = const.tile([1, HALO], BF16, tag="valid")
        for et in range(8):
            nc.sync.dma_start(wv_t[et][:], wv_d[et * 128:(et + 1) * 128, :])
        for db in range(1, 8):
            nc.sync.dma_start(wq_t[db][:], wq_d[db])
            nc.sync.dma_start(wk_t[db][:], wk_d[db])
        nc.gpsimd.dma_start(wq_t[0][:], wq_d[0])
        nc.gpsimd.dma_start(wk_t[0][:], wk_d[0])
        nc.gpsimd.dma_start(bq_sb[:], bq_d[:, :])
        nc.gpsimd.dma_start(bk_sb[:], bk_d[:, :])
        nc.gpsimd.dma_start(valid_sb[:], xvalid_d[:, :])
        nc.gpsimd.dma_start(m4[:], m4_d[:, :])
        nc.gpsimd.dma_start(bc_sb[:].rearrange("o h d -> o (h d)"), bc_d[:, :])
        for et in range(8):
            nc.gpsimd.dma_start(wo_t[et][:], wo_d[et * 128:(et + 1) * 128, :])
        nc.gpsimd.dma_start(bo_sb[:], bo_d[:, :])
        ones_sb = const.tile([1, 128], BF16, tag="ones")
        nc.vector.memset(ones_sb[:], 1.0)

        # ---- persistent activations ---------------------------------------
        QT = pers.tile([128, 8, R], BF16, tag="QT")          # [d_p, d_t, q]
        KT = pers.tile([128, 8, HALO], BF16, tag="KT")       # [d_p, d_t, seq]
        Vaug = pers.tile([128, 3, H, D + 1], BF16, tag="Vaug")
        Asc = pers.tile([128, NQB, H * D], BF16, tag="Asc")  # [q_p, qblk, dims]
        AT = pers.tile([128, 8, R], BF16, tag="AT")          # [d_p, d_t, q]

        # ---- V projection, et-outer so it paces with the Wv DMA stream ----
        vps = [ps([128, 512]) for _ in range(6)]             # [st*2+hf]
        for et in range(8):
            for st in range(3):
                for hf in range(2):
                    nc.tensor.matmul(vps[st * 2 + hf][:],
                                     xT[:, et, st * 128:(st + 1) * 128],
                                     wv_t[et][:, hf * 512:(hf + 1) * 512],
                                     start=(et == 0), stop=(et == 7))
        for st in range(3):
            for hf in range(2):
                src = vps[st * 2 + hf][:].rearrange("p (h d) -> p h d", d=D)
                nc.scalar.copy(Vaug[:, st, hf * 8:(hf + 1) * 8, 0:D], src)
        nc.vector.memset(Vaug[:, :, :, D:D + 1], 1.0)

        # ---- fused projections + banded attention, one head-pair at a time
        # round r = db (one head pair, BOTH query blocks). Emission order:
        #   1. Q^T/K^T projection matmuls for db
        #   2. PV matmuls + epilogue of round r-1 (p tiles ready), including
        #      the Asc -> AT DMA transpose for the finished head pair
        #   3. S matmuls (one [128, 512] psum per head = both qblk/cblk
        #      quadrants) + fused exp/-1/mask chain for round r
        # Per-head p layout: [q0c0 | q0c1 | q1c0 | q1c1], quadrant j uses
        # keys halo block (qblk+cblk) and mask m0/m1 alternating.
        prev = None  # (db, ptiles{h: pt})

        wqv = [wq_t[db][:].rearrange("p (t d) -> p t d", d=128) for db in range(8)]
        wkv = [wk_t[db][:].rearrange("p (t d) -> p t d", d=128) for db in range(8)]

        def proj(db):
            qp = ps([128, R])
            for et in range(8):
                nc.tensor.matmul(qp[:], wqv[db][:, et, :],
                                 xT[:, et, 64:64 + R],
                                 start=(et == 0), stop=(et == 7))
            nc.scalar.add(QT[:, db, :], qp[:], bq_sb[:, db:db + 1])
            kp = ps([128, HALO])
            for et in range(8):
                nc.tensor.matmul(kp[:], wkv[db][:, et, :],
                                 xT[:, et, :], start=(et == 0), stop=False)
            nc.tensor.matmul(kp[:], bk_sb[0:1, db * 128:(db + 1) * 128],
                             valid_sb[0:1, :], start=False, stop=True)
            nc.scalar.copy(KT[:, db, :], kp[:])

        def pv_flush(pr):
            db, ptl = pr
            pvs = {}
            for qblk in range(NQB):
                pvs[qblk] = ps([128, 2 * (D + 1)])
            for qblk in range(NQB):
                pv = pvs[qblk]
                for i, h in enumerate((2 * db, 2 * db + 1)):
                    off = i * (D + 1)
                    for cblk in range(2):
                        quad = qblk * 2 + cblk
                        nc.tensor.matmul(pv[:, off:off + D + 1],
                                         ptl[h][:, quad * 128:(quad + 1) * 128],
                                         Vaug[:, qblk + cblk, h, :],
                                         start=(i == 0 and cblk == 0),
                                         stop=False)
            for qblk in range(NQB):
                pv = pvs[qblk]
                bc_pair = bc_sb[0:1, 2 * db:2 * db + 2, :]
                nc.tensor.matmul(pv[:, 0:2 * (D + 1)], ones_sb[0:1, :],
                                 bc_pair.rearrange("o h d -> o (h d)"),
                                 start=False, stop=True)
            for qblk in range(NQB):
                pv = pvs[qblk]
                for i, h in enumerate((2 * db, 2 * db + 1)):
                    off = i * (D + 1)
                    zinv = zpool.tile([128, 1], F32, tag="z", name="zinv")
                    nc.vector.reciprocal(zinv[:], pv[:, off + D:off + D + 1])
                    nc.scalar.activation(Asc[:, qblk, h * D:(h + 1) * D],
                                         pv[:, off:off + D],
                                         mybir.ActivationFunctionType.Copy,
                                         scale=zinv[:])
            for qblk in range(NQB):
                nc.scalar.dma_start(AT[:, db, qblk * 128:(qblk + 1) * 128],
                                    Asc[:, qblk, db * 128:(db + 1) * 128],
                                    transpose=True)

        for r in range(8 + 1):
            if r < 8:
                db = r
                proj(db)
                if prev is not None:
                    pv_flush(prev)
                # S matmuls: head A on PE rows 0-63, head B on rows 64-127 —
                # interleaved emission so the two row-groups run concurrently.
                sps = {h: ps([128, 512]) for h in (2 * db, 2 * db + 1)}
                for quad in range(4):
                    qblk, cblk = quad // 2, quad % 2
                    for i, h in enumerate((2 * db, 2 * db + 1)):
                        rr = i * 64
                        nc.tensor.matmul(
                            sps[h][:, quad * 128:(quad + 1) * 128],
                            KT[rr:rr + 64, db,
                               (qblk + cblk) * 128:(qblk + cblk + 1) * 128],
                            QT[rr:rr + 64, db, qblk * 128:(qblk + 1) * 128],
                            start=(quad == 0), stop=(quad == 3))
                ptl = {}
                for h in (2 * db, 2 * db + 1):
                    et_ = epool.tile([128, 512], F32, tag="e", name="et_")
                    nc.scalar.activation(et_[:], sps[h][:],
                                         mybir.ActivationFunctionType.Exp)
                    pt = ppool.tile([128, 512], BF16, tag="p", name="pt")
                    nc.vector.scalar_tensor_tensor(
                        pt[:], et_[:], -1.0, m4[:],
                        mybir.AluOpType.add, mybir.AluOpType.mult)
                    ptl[h] = pt
                prev = (db, ptl)
            else:
                pv_flush(prev)

        # ---- output projection: O = A @ Wo + bo' --------------------------
        ops = [ps([128, 512]) for _ in range(2 * NQB)]
        for at in range(8):
            for qblk in range(NQB):
                for hf in range(2):
                    nc.tensor.matmul(ops[qblk * 2 + hf][:],
                                     AT[:, at, qblk * 128:(qblk + 1) * 128],
                                     wo_t[at][:, hf * 512:(hf + 1) * 512],
                                     start=(at == 0), stop=False)
        for qblk in range(NQB):
            for hf in range(2):
                nc.tensor.matmul(ops[qblk * 2 + hf][:], ones_sb[0:1, :],
                                 bo_sb[0:1, hf * 512:(hf + 1) * 512],
                                 start=False, stop=True)
        for qblk in range(NQB):
            ob = obpool.tile([128, E], BF16, tag="ob")
            for hf in range(2):
                nc.vector.tensor_copy(ob[:, hf * 512:(hf + 1) * 512],
                                      ops[qblk * 2 + hf][:])
            nc.scalar.dma_start(out_d[qblk * 128:(qblk + 1) * 128, :], ob[:])

    nc.compile()
    return nc


_NC = None


def get_nc():
    global _NC
    if _NC is None:
        _NC = build_graph()
    return _NC


def make_in_maps(x, Wq, bq, Wk, bk, Wv, bv, Wo, bo):
    f = lambda a: np.ascontiguousarray(np.asarray(a, dtype=np.float32))
    bf = lambda a: np.ascontiguousarray(
        np.asarray(a, dtype=np.float32).astype(NPBF16))
    x2 = f(x).reshape(N, E)
    Wv32, Wo32 = f(Wv), f(Wo)
    ci = np.arange(128, dtype=np.float32)[:, None]  # key index c (partitions)
    qi = np.arange(128, dtype=np.float32)[None, :]  # query index q (free)
    m0 = (ci >= qi).astype(np.float32)
    m1 = (ci <= qi).astype(np.float32)
    mask4 = np.concatenate([m0, m1, m0, m1], axis=1)
    # host-folded epilogue bias: bo' = bo + bv @ Wo
    bo_row = (f(bo) + f(bv) @ Wo32).reshape(1, E)
    # host-computed global-sum row: per head [sum_n V_n | N]
    sv = (x2.sum(0, dtype=np.float32) @ Wv32).reshape(H, D)
    biascat = np.concatenate(
        [sv, np.full((H, 1), float(N), np.float32)], axis=1).reshape(1, -1)
    # db-major Wq/Wk: dbm[db, e_part, et*128+d] = W[et*128+e_part, db*128+d]
    def dbm(W):
        return np.ascontiguousarray(
            f(W).reshape(8, 128, 8, 128).transpose(2, 1, 0, 3)
            .reshape(8, 128, H * D).astype(NPBF16))
    common = {
        "Wq_db": dbm(Wq), "Wk_db": dbm(Wk), "Wv": bf(Wv), "Wo": bf(Wo),
        "bq_r": f(bq).reshape(8, 128).T.copy(),
        "bk_row": bf(bk).reshape(1, H * D),
        "bo_row": bf(bo_row),
        "biascat_r": bf(biascat),
        "mask4": bf(mask4),
    }
    in_maps = []
    for c in range(8):
        r0 = c * R
        xh = np.zeros((HALO, E), np.float32)
        valid = np.zeros((1, HALO), NPBF16)
        lo, hi = r0 - 64, r0 + R + 64
        slo, shi = max(lo, 0), min(hi, N)
        xh[slo - lo: shi - lo] = x2[slo:shi]
        valid[0, slo - lo: shi - lo] = 1.0
        xhT = np.ascontiguousarray(xh.T.astype(NPBF16))
        in_maps.append({**common, "xhT": xhT, "xvalid": valid})
    return in_maps


def kernel(x, Wq, bq, Wk, bk, Wv, bv, Wo, bo, _trace=False, _trace_kwargs=None):
    nc = get_nc()
    in_maps = make_in_maps(x, Wq, bq, Wk, bk, Wv, bv, Wo, bo)
    res = run_bass_kernel_spmd(nc, in_maps, list(range(8)), trace=_trace,
                               **(_trace_kwargs or {}))
    out = np.concatenate([res.results[c]["out"] for c in range(8)], axis=0)
    kernel.last_result = res
    return out[None].astype(np.float32)
